# revision 22
# baseline (speedup 1.0000x reference)
"""DPOT2D layer (AFNO-style) Trainium2 kernel.

out = x + irfft2_pad(blockMLP(trunc64(rfft2(x))))   (ortho norm)

Sharding: tensor-parallel over the 8 block-diagonal channel groups — core n
gets channels [n*64, (n+1)*64) and its block's MLP weights. Blocks never mix,
so there is zero cross-core communication.

Per core, every FFT stage is a DFT matmul on the TensorEngine (bf16 operands,
fp32 PSUM accumulation), with PE-transpose corner turns between stages:

  A:  U[k1s,(w,c)]    = F_h^T  @ x          (contract h, 2x128 K-chunks)
  t1: V[w,(c,k1s)]    = corner turn of U    (4 transposes / PSUM bank)
  B:  Y[k2s,(c,k1)]   = DFT_w on complex U  (re/im column accumulation)
  t2: Yt[(s,c),(k1,k2)] = s-stacked corner turn of Y (16 transposes / bank)
  L1: o1 = gelu(M1' Yt + b1)                (K=128 single matmul per tile)
  L2: O2[(o2r|o2i),(k1,k2)] = M2 o1 + b2    (K=128)
  t3: R[(s,k2),(k1,o)] = s-stacked corner turn of O2
  iW: G[w,(j,k1,c)]   = hermitian irfft_w matmuls (K=128 single matmul)
  t4: Ght[k1s,(w,c)]  = corner turn of G
  iH: x'[h,(w,c)]     = E_h^T @ Ght + x_bf16 (DVE tensor add), DMA out bf16

DMA traffic per core: 16.8 MB in (bf16 x, kept resident per batch for the
residual) + 16.8 MB out (bf16, host upcasts to f32).
"""

import numpy as np
import ml_dtypes

import concourse.bass as bass
import concourse.mybir as mybir
from concourse import bacc
from concourse import masks
from concourse.tile import TileContext
from concourse.bass_utils import run_bass_kernel_spmd

B = 2
H = 256
W = 256
C = 512
NB = 8
BS = 64          # channels per block (= per core)
KEEP = 64        # kept modes per spatial dim
HID = 128
P = 128

BF16 = mybir.dt.bfloat16
F32 = mybir.dt.float32
AF = mybir.ActivationFunctionType

_CACHED_NC = None


def _host_consts():
    """DFT matrices shared by all cores (fp32 -> bf16)."""
    h = np.arange(H, dtype=np.float64)[:, None]
    k = np.arange(KEEP, dtype=np.float64)[None, :]
    th = 2.0 * np.pi * h * k / H
    F = np.concatenate([np.cos(th), -np.sin(th)], axis=1) / 16.0      # (256,128)
    Fwre, Fwim = F[:, :KEEP], F[:, KEEP:]
    lb_re = np.concatenate([Fwre, Fwim], axis=1)                      # (256,128)
    lb_im = np.concatenate([-Fwim, Fwre], axis=1)
    alpha = np.where(np.arange(KEEP) == 0, 1.0, 2.0)
    k2 = np.arange(KEEP, dtype=np.float64)[:, None]
    wv = np.arange(W, dtype=np.float64)[None, :]
    tw = 2.0 * np.pi * k2 * wv / W
    Ca = alpha[:, None] * np.cos(tw) / 16.0                           # (64,256)
    Sa = alpha[:, None] * np.sin(tw) / 16.0
    k1 = np.arange(KEEP, dtype=np.float64)[:, None]
    hv = np.arange(H, dtype=np.float64)[None, :]
    tih = 2.0 * np.pi * k1 * hv / H
    Ehc = np.cos(tih) / 16.0                                          # (64,256)
    Ehs = np.sin(tih) / 16.0
    lih_full = np.concatenate([Ehc, -Ehs], axis=0)                    # (128,256)

    bf = ml_dtypes.bfloat16
    ffwd = np.stack([F[0:128], F[128:256]]).astype(bf)                # (2,128,128)
    lbw = np.stack([
        np.stack([lb_re[0:128], lb_im[0:128]]),
        np.stack([lb_re[128:256], lb_im[128:256]]),
    ]).astype(bf)                                                     # (2,2,128,128)
    # s-stacked inverse-W weights: rows = (s 2, k2 64) = 128
    # j=0 (Gre): [Ca; -Sa], j=1 (Gim): [Sa; Ca]
    liw = np.stack([
        np.stack([np.concatenate([Ca[:, 0:128], -Sa[:, 0:128]], axis=0),
                  np.concatenate([Sa[:, 0:128], Ca[:, 0:128]], axis=0)]),
        np.stack([np.concatenate([Ca[:, 128:256], -Sa[:, 128:256]], axis=0),
                  np.concatenate([Sa[:, 128:256], Ca[:, 128:256]], axis=0)]),
    ]).astype(bf)                                                     # (2,2,128,128)
    lih = np.stack([lih_full[:, 0:128], lih_full[:, 128:256]]).astype(bf)  # (2,128,128)
    return ffwd, lbw, liw, lih


def _build_nc(loop_iters=0, probe=None):
    """loop_iters>0 wraps the whole per-batch pipeline in an on-device
    For_i repeat loop — used only by the timing harness to amortize the
    ~80ms axon dispatch overhead out of the measurement.
    probe: None | 'dma' (DMAs only) | 'compute' (no input/residual DMAs)."""
    nc = bacc.Bacc()

    xbf = nc.declare_dram_parameter("xbf", [B, H, W, BS], BF16, isOutput=False)
    ffwd_d = nc.declare_dram_parameter("ffwd", [2, P, P], BF16, isOutput=False)
    lbw_d = nc.declare_dram_parameter("lbw", [2, 2, P, P], BF16, isOutput=False)
    m1_d = nc.declare_dram_parameter("m1", [2, P, HID], BF16, isOutput=False)
    m2_d = nc.declare_dram_parameter("m2", [2, HID, P], BF16, isOutput=False)
    b1s_d = nc.declare_dram_parameter("b1s", [2, HID, 1], F32, isOutput=False)
    b2s_d = nc.declare_dram_parameter("b2s", [P, 1], F32, isOutput=False)
    liw_d = nc.declare_dram_parameter("liw", [2, 2, P, P], BF16, isOutput=False)
    lih_d = nc.declare_dram_parameter("lih", [2, P, P], BF16, isOutput=False)
    out = nc.declare_dram_parameter("out", [B, H, W, BS], BF16, isOutput=True)

    with TileContext(nc) as tc:
        consts = tc.alloc_tile_pool(name="consts", bufs=1)
        ident = consts.tile([P, P], BF16, name="ident")
        masks.make_identity(nc, ident[:])

        def const2d(name, dram_ap, shape, dtype=BF16):
            t = consts.tile(shape, dtype, name=name)
            nc.sync.dma_start(out=t[:], in_=dram_ap)
            return t

        FW = [const2d(f"fw{hh}", ffwd_d[hh], [P, P]) for hh in range(2)]
        LBW = [[const2d(f"lbw{wh}{s}", lbw_d[wh, s], [P, P]) for s in range(2)]
               for wh in range(2)]
        M1 = [const2d(f"m1_{j}", m1_d[j], [P, HID]) for j in range(2)]
        M2 = [const2d(f"m2_{s}", m2_d[s], [HID, P]) for s in range(2)]
        LIW = [[const2d(f"liw{wh}{j}", liw_d[wh, j], [P, P])
                for j in range(2)] for wh in range(2)]
        LIH = [const2d(f"lih{hc}", lih_d[hc], [P, P]) for hc in range(2)]
        b1s_t = [const2d(f"b1s{j}", b1s_d[j], [HID, 1], F32) for j in range(2)]
        b2s_t = const2d("b2s", b2s_d[:], [P, 1], F32)

        # PSUM eviction engine choice (PSUM-capable engines: DVE + ACT).
        # f32-source copies run 1x everywhere -> ACT (1.2 GHz beats DVE
        # 0.96); all-bf16 unit-stride copies hit DVE's 2x packed mode.
        def cp_act(dst, src):
            nc.scalar.activation(out=dst, in_=src, func=AF.Copy)

        def cp_dve(dst, src):
            nc.vector.tensor_copy(out=dst, in_=src)

        # Tag-sharing across stage lifetimes keeps SBUF within budget:
        #   tagA/tagB: U[wh] -> G[wh]   tagC/tagD: V[wh] -> Ght[wh]
        #   tagE: Y -> R                tagF: Yt -> O2
        sb = tc.alloc_tile_pool(name="sb", bufs=1)
        xin = tc.alloc_tile_pool(name="xin", bufs=1)
        outp = tc.alloc_tile_pool(name="outp", bufs=4)
        pmm = tc.alloc_tile_pool(name="pmm", bufs=3, space="PSUM")
        ptp = tc.alloc_tile_pool(name="ptp", bufs=2, space="PSUM")

        import contextlib
        loop_ctx = tc.For_i(0, loop_iters, 1) if loop_iters else contextlib.nullcontext()
        with loop_ctx:
            if probe == "dma":
                _emit_dma_probe(nc, tc, locals())
            else:
                _emit_body(nc, tc, locals(), skip_dma=(probe == "compute"))
        ptp.release()
        pmm.release()
        outp.release()
        xin.release()
        sb.release()
        consts.release()
    nc.compile()
    return nc


def _emit_dma_probe(nc, tc, env):
    """Same DMA traffic as the real kernel (x-in bf16, out bf16), no compute."""
    xbf = env["xbf"]; out = env["out"]
    xin = env["xin"]; outp = env["outp"]
    for b in range(B):
        for wc in range(8):
            for hh in range(2):
                t = xin.tile([P, 32, BS], BF16, tag=f"xin{hh}_{wc}",
                             name=f"pxin{hh}_{b}_{wc}")
                nc.sync.dma_start(
                    out=t[:],
                    in_=xbf[b, hh * P:(hh + 1) * P, wc * 32:(wc + 1) * 32, :])
                ot = outp.tile([P, 32, BS], BF16, tag="ot",
                               name=f"pot_{b}_{hh}_{wc}")
                nc.vector.tensor_copy(out=ot[:], in_=t[:])
                nc.sync.dma_start(
                    out=out[b, hh * P:(hh + 1) * P, wc * 32:(wc + 1) * 32, :],
                    in_=ot[:])


def _emit_body(nc, tc, env, skip_dma=False):
    xbf = env["xbf"]; out = env["out"]
    FW = env["FW"]; LBW = env["LBW"]; M1 = env["M1"]; M2 = env["M2"]
    LIW = env["LIW"]; LIH = env["LIH"]; b1s_t = env["b1s_t"]; b2s_t = env["b2s_t"]
    ident = env["ident"]; cp_act = env["cp_act"]; cp_dve = env["cp_dve"]
    sb = env["sb"]; xin = env["xin"]; outp = env["outp"]
    pmm = env["pmm"]; ptp = env["ptp"]

    for b in range(B):
        # ---------------- stage A: U[wh] (128=k1s, (w 128, c 64)) ----------
        # xin tiles stay resident for the whole batch; iH reads them as the
        # residual.
        xt_all = {}
        U = [sb.tile([P, 128, BS], BF16, tag=f"tagAB{wh}", name=f"U{wh}_{b}")
             for wh in range(2)]
        for wc in range(8):          # w chunks of 32
            for hh in range(2):
                t = xin.tile([P, 32, BS], BF16, tag=f"xin{hh}_{wc}",
                             bufs=(2 if wc < 5 else 1),
                             name=f"xin{hh}_{b}_{wc}")
                if not skip_dma:
                    nc.sync.dma_start(
                        out=t[:],
                        in_=xbf[b, hh * P:(hh + 1) * P, wc * 32:(wc + 1) * 32, :])
                else:
                    nc.sync.dma_start(
                        out=t[0:1, 0:1, :],
                        in_=xbf[b, 0:1, 0:1, :])
                xt_all[(hh, wc)] = t
            for np2 in range(2):     # pairs of N=512 pieces -> FD=1024 evict
                ps = pmm.tile([P, 16, BS], F32, tag="mm", name=f"psA_{b}_{wc}_{np2}")
                for i in range(2):
                    nn = np2 * 2 + i
                    nc.tensor.matmul(ps[:, i * 8:(i + 1) * 8, :], FW[0],
                                     xt_all[(0, wc)][:, nn * 8:(nn + 1) * 8, :],
                                     start=True, stop=False)
                    nc.tensor.matmul(ps[:, i * 8:(i + 1) * 8, :], FW[1],
                                     xt_all[(1, wc)][:, nn * 8:(nn + 1) * 8, :],
                                     start=False, stop=True)
                wg = wc * 4 + np2 * 2   # global 8-w group index (0..31)
                cp_act(U[wg // 16][:, (wg % 16) * 8:(wg % 16) * 8 + 16, :], ps[:])

        # ---------------- turn1: V[wh] (128=w, (c 64, k1s 128)) ------------
        # 4 transposes share one PSUM tile -> single FD=512 eviction.
        V = [sb.tile([P, BS, P], BF16, tag=f"tagCD{wh}", name=f"V{wh}_{b}")
             for wh in range(2)]
        for wh in range(2):
            for cb in range(8):      # blocks of 8 c
                pt = ptp.tile([P, 8, P], BF16, tag="tp", name=f"t1_{b}_{wh}_{cb}")
                for i in range(8):
                    nc.tensor.transpose(pt[:, i, :], U[wh][:, :, cb * 8 + i],
                                        ident[:])
                cp_dve(V[wh][:, cb * 8:(cb + 1) * 8, :], pt[:])

        # ---------------- stage B: Y (128=k2s, (c 64, k1 64)) --------------
        Y = sb.tile([P, BS, KEEP], BF16, tag="tagE", name=f"Y_{b}")
        for np2 in range(4):         # pairs of 8-c chunks -> FD=1024 evict
            ps = pmm.tile([P, 16, KEEP], F32, tag="mm", name=f"psB_{b}_{np2}")
            for i in range(2):
                nn = np2 * 2 + i
                first = True
                for wh in range(2):
                    for s in range(2):   # 0: re cols (k1s 0:64), 1: im cols
                        rhs = V[wh][:, nn * 8:(nn + 1) * 8,
                                    s * KEEP:(s + 1) * KEEP]
                        nc.tensor.matmul(ps[:, i * 8:(i + 1) * 8, :],
                                         LBW[wh][s], rhs,
                                         start=first, stop=(wh == 1 and s == 1))
                        first = False
            cp_act(Y[:, np2 * 16:(np2 + 1) * 16, :], ps[:])

        # ------- turn2: Yt ((s,c) 128, (k1 64, k2 64)) s-stacked -----------
        # Yt[64s+c, k1, k2] = Y[64s+k2, c, k1]; 16 transposes / PSUM tile.
        Yt = sb.tile([P, KEEP, KEEP], BF16, tag="tagF", name=f"Yt_{b}")
        for k1b in range(4):
            pt = ptp.tile([P, 16, KEEP], BF16, tag="tp", name=f"t2_{b}_{k1b}")
            for kk in range(16):
                k1 = k1b * 16 + kk
                for s in range(2):
                    nc.tensor.transpose(
                        pt[s * KEEP:(s + 1) * KEEP, kk, :],
                        Y[s * KEEP:(s + 1) * KEEP, :, k1],
                        ident[s * KEEP:(s + 1) * KEEP, s * KEEP:(s + 1) * KEEP])
            cp_dve(Yt[:, k1b * 16:(k1b + 1) * 16, :], pt[:])

        # ---------------- MLP L1 (K=128) + gelu ----------------------------
        o1 = [sb.tile([HID, KEEP, KEEP], BF16, tag=f"tagCD{j}", name=f"o1_{j}_{b}")
              for j in range(2)]
        for j in range(2):
            for k1b in range(4):     # 16 k1 per chunk -> FD=1024 gelu
                ps = pmm.tile([HID, 16, KEEP], F32, tag="mm",
                              name=f"ps1_{b}_{j}_{k1b}")
                for i in range(2):
                    nc.tensor.matmul(
                        ps[:, i * 8:(i + 1) * 8, :], M1[j],
                        Yt[:, k1b * 16 + i * 8:k1b * 16 + (i + 1) * 8, :],
                        start=True, stop=True)
                nc.scalar.activation(out=o1[j][:, k1b * 16:(k1b + 1) * 16, :],
                                     in_=ps[:], func=AF.Gelu, bias=b1s_t[j][:])

        # ---------------- MLP L2 (K=128) + bias ----------------------------
        O2 = sb.tile([P, KEEP, KEEP], BF16, tag="tagF", name=f"O2_{b}")
        for k1b in range(4):
            ps = pmm.tile([P, 16, KEEP], F32, tag="mm", name=f"ps2_{b}_{k1b}")
            for i in range(2):
                sl = slice(k1b * 16 + i * 8, k1b * 16 + (i + 1) * 8)
                nc.tensor.matmul(ps[:, i * 8:(i + 1) * 8, :], M2[0],
                                 o1[0][:, sl, :], start=True, stop=False)
                nc.tensor.matmul(ps[:, i * 8:(i + 1) * 8, :], M2[1],
                                 o1[1][:, sl, :], start=False, stop=True)
            nc.scalar.activation(out=O2[:, k1b * 16:(k1b + 1) * 16, :],
                                 in_=ps[:], func=AF.Identity, bias=b2s_t[:])

        # ------- turn3: R ((s,k2) 128, (k1 64, o 64)) s-stacked ------------
        # R[64s+k2, k1, o] = O2[64s+o, k1, k2]
        R = sb.tile([P, KEEP, KEEP], BF16, tag="tagE", name=f"R_{b}")
        for k1b in range(4):
            pt = ptp.tile([P, 16, KEEP], BF16, tag="tp", name=f"t3_{b}_{k1b}")
            for kk in range(16):
                k1 = k1b * 16 + kk
                for s in range(2):
                    nc.tensor.transpose(
                        pt[s * KEEP:(s + 1) * KEEP, kk, :],
                        O2[s * KEEP:(s + 1) * KEEP, k1, :],
                        ident[s * KEEP:(s + 1) * KEEP, s * KEEP:(s + 1) * KEEP])
            cp_dve(R[:, k1b * 16:(k1b + 1) * 16, :], pt[:])

        # ---------------- invW: G[wh] (128=w, (j 2, k1 64, c' 64)) ---------
        G = [sb.tile([P, 2, KEEP, BS], BF16, tag=f"tagAB{wh}", name=f"G{wh}_{b}")
             for wh in range(2)]
        for wh in range(2):
            for j in range(2):       # 0: Gre, 1: Gim
                for k1b in range(4):
                    ps = pmm.tile([P, 16, BS], F32, tag="mm",
                                  name=f"psW_{b}_{wh}_{j}_{k1b}")
                    for i in range(2):
                        sl = slice(k1b * 16 + i * 8, k1b * 16 + (i + 1) * 8)
                        nc.tensor.matmul(ps[:, i * 8:(i + 1) * 8, :],
                                         LIW[wh][j], R[:, sl, :],
                                         start=True, stop=True)
                    (cp_act if k1b % 2 == 0 else cp_dve)(
                        G[wh][:, j, k1b * 16:(k1b + 1) * 16, :], ps[:])

        # ---------------- turn4: Ght (128=k1s, (w 128, c' 64)) -------------
        # 4 transposes / PSUM tile; eviction is a strided (1x) copy because
        # PSUM holds (c, w) but Ght needs (w, c).
        Ght = [sb.tile([P, P, BS], BF16, tag=f"tagCD{wh}", name=f"Ght{wh}_{b}")
               for wh in range(2)]
        for wh in range(2):
            for cb in range(8):
                pt = ptp.tile([P, 8, P], BF16, tag="tp", name=f"t4_{b}_{wh}_{cb}")
                for i in range(8):
                    # free slice (j 2, k1 64) -> out partitions [k1re | k1im]
                    nc.tensor.transpose(pt[:, i, :],
                                        G[wh][:, :, :, cb * 8 + i], ident[:])
                (cp_dve if cb % 2 == 0 else cp_act)(
                    Ght[wh][:, :, cb * 8:(cb + 1) * 8],
                    pt.transpose([0, 2, 1]))

        # ---------------- invH + residual + store --------------------------
        for q8 in range(8):          # groups of 32 w
            for hc in range(2):
                ot = outp.tile([P, 32, BS], BF16, tag="ot",
                               name=f"ot_{b}_{hc}_{q8}")
                for np2 in range(2):  # pairs of N=512 pieces -> FD=1024
                    ps = pmm.tile([P, 16, BS], F32, tag="mm",
                                  name=f"psH_{b}_{hc}_{q8}_{np2}")
                    for i in range(2):
                        wg = q8 * 4 + np2 * 2 + i   # global 8-w group (0..31)
                        nc.tensor.matmul(
                            ps[:, i * 8:(i + 1) * 8, :], LIH[hc],
                            Ght[wg // 16][:, (wg % 16) * 8:(wg % 16) * 8 + 8, :],
                            start=True, stop=True)
                    osl = ot[:, np2 * 16:(np2 + 1) * 16, :]
                    xsl = xt_all[(hc, q8)][:, np2 * 16:(np2 + 1) * 16, :]
                    if np2 == 0:
                        # direct PSUM-side add on DVE (f32 src, 1x)
                        nc.vector.tensor_add(out=osl, in0=ps[:], in1=xsl)
                    else:
                        # ACT evicts PSUM -> bf16, DVE adds all-bf16 (2x)
                        nc.scalar.activation(out=osl, in_=ps[:], func=AF.Copy)
                        nc.vector.tensor_add(out=osl, in0=osl, in1=xsl)
                if not skip_dma:
                    nc.sync.dma_start(
                        out=out[b, hc * P:(hc + 1) * P,
                                q8 * 32:(q8 + 1) * 32, :],
                        in_=ot[:])
                else:
                    nc.sync.dma_start(
                        out=out[b, 0:1, 0:1, :],
                        in_=ot[0:1, 0:1, :])


def _prepare_in_maps(x, w1, b1, w2, b2):
    bf = ml_dtypes.bfloat16
    ffwd, lbw, liw, lih = _host_consts()
    x = np.asarray(x, dtype=np.float32)

    in_maps = []
    for n in range(NB):
        xs = np.ascontiguousarray(x[..., n * BS:(n + 1) * BS])
        w1n = np.asarray(w1[:, n], dtype=np.float32)   # (2,64,128)
        w2n = np.asarray(w2[:, n], dtype=np.float32)   # (2,128,64)
        b1n = np.asarray(b1[:, n], dtype=np.float32)   # (2,128)
        b2n = np.asarray(b2[:, n], dtype=np.float32)   # (2,64)
        # s-stacked L1 weights: rows = (s 2, c 64) = 128
        m1 = np.stack([
            np.concatenate([w1n[0], -w1n[1]], axis=0),
            np.concatenate([w1n[1], w1n[0]], axis=0),
        ]).astype(bf)                                   # (2,128,128)
        m2 = np.stack([
            np.concatenate([w2n[0], w2n[1]], axis=1),
            np.concatenate([-w2n[1], w2n[0]], axis=1),
        ]).astype(bf)                                   # (2,128,128)
        in_maps.append({
            "xbf": xs.astype(bf),
            "ffwd": ffwd,
            "lbw": lbw,
            "m1": m1,
            "m2": m2,
            "b1s": b1n[:, :, None].copy(),
            "b2s": np.concatenate([b2n[0], b2n[1]])[:, None].copy(),
            "liw": liw,
            "lih": lih,
        })

    return in_maps


def kernel(x, w1, b1, w2, b2):
    global _CACHED_NC
    if _CACHED_NC is None:
        _CACHED_NC = _build_nc()
    nc = _CACHED_NC
    in_maps = _prepare_in_maps(x, w1, b1, w2, b2)
    res = run_bass_kernel_spmd(nc, in_maps, list(range(NB)))
    return np.concatenate(
        [res.results[i]["out"].astype(np.float32) for i in range(NB)], axis=-1)


# revision 23
# speedup vs baseline: 1.0764x; 1.0764x over previous
"""DPOT2D layer (AFNO-style) Trainium2 kernel.

out = x + irfft2_pad(blockMLP(trunc64(rfft2(x))))   (ortho norm)

Sharding: tensor-parallel over the 8 block-diagonal channel groups — core n
gets channels [n*64, (n+1)*64) and its block's MLP weights. Blocks never mix,
so there is zero cross-core communication.

Per core, every FFT stage is a DFT matmul on the TensorEngine (bf16 operands,
fp32 PSUM accumulation), with PE-transpose corner turns between stages:

  A:  U[k1s,(w,c)]    = F_h^T  @ x          (contract h, 2x128 K-chunks)
  t1: V[w,(c,k1s)]    = corner turn of U    (4 transposes / PSUM bank)
  B:  Y[k2s,(c,k1)]   = DFT_w on complex U  (re/im column accumulation)
  t2: Yt[(s,c),(k1,k2)] = s-stacked corner turn of Y (16 transposes / bank)
  L1: o1 = gelu(M1' Yt + b1)                (K=128 single matmul per tile)
  L2: O2[(o2r|o2i),(k1,k2)] = M2 o1 + b2    (K=128)
  t3: R[(s,k2),(k1,o)] = s-stacked corner turn of O2
  iW: G[w,(j,k1,c)]   = hermitian irfft_w matmuls (K=128 single matmul)
  t4: Ght[k1s,(w,c)]  = corner turn of G
  iH: x'[h,(w,c)]     = E_h^T @ Ght + x_bf16 (DVE tensor add), DMA out bf16

DMA traffic per core: 16.8 MB in (bf16 x, kept resident per batch for the
residual) + 16.8 MB out (bf16, host upcasts to f32).
"""

import numpy as np
import ml_dtypes

import concourse.bass as bass
import concourse.mybir as mybir
from concourse import bacc
from concourse import masks
from concourse.tile import TileContext
from concourse.bass_utils import run_bass_kernel_spmd

B = 2
H = 256
W = 256
C = 512
NB = 8
BS = 64          # channels per block (= per core)
KEEP = 64        # kept modes per spatial dim
HID = 128
P = 128

BF16 = mybir.dt.bfloat16
F32 = mybir.dt.float32
AF = mybir.ActivationFunctionType

_CACHED_NC = None


def _host_consts():
    """DFT matrices shared by all cores (fp32 -> bf16)."""
    h = np.arange(H, dtype=np.float64)[:, None]
    k = np.arange(KEEP, dtype=np.float64)[None, :]
    th = 2.0 * np.pi * h * k / H
    F = np.concatenate([np.cos(th), -np.sin(th)], axis=1) / 16.0      # (256,128)
    Fwre, Fwim = F[:, :KEEP], F[:, KEEP:]
    lb_re = np.concatenate([Fwre, Fwim], axis=1)                      # (256,128)
    lb_im = np.concatenate([-Fwim, Fwre], axis=1)
    alpha = np.where(np.arange(KEEP) == 0, 1.0, 2.0)
    k2 = np.arange(KEEP, dtype=np.float64)[:, None]
    wv = np.arange(W, dtype=np.float64)[None, :]
    tw = 2.0 * np.pi * k2 * wv / W
    Ca = alpha[:, None] * np.cos(tw) / 16.0                           # (64,256)
    Sa = alpha[:, None] * np.sin(tw) / 16.0
    k1 = np.arange(KEEP, dtype=np.float64)[:, None]
    hv = np.arange(H, dtype=np.float64)[None, :]
    tih = 2.0 * np.pi * k1 * hv / H
    Ehc = np.cos(tih) / 16.0                                          # (64,256)
    Ehs = np.sin(tih) / 16.0
    lih_full = np.concatenate([Ehc, -Ehs], axis=0)                    # (128,256)

    bf = ml_dtypes.bfloat16
    ffwd = np.stack([F[0:128], F[128:256]]).astype(bf)                # (2,128,128)
    lbw = np.stack([
        np.stack([lb_re[0:128], lb_im[0:128]]),
        np.stack([lb_re[128:256], lb_im[128:256]]),
    ]).astype(bf)                                                     # (2,2,128,128)
    # s-stacked inverse-W weights: rows = (s 2, k2 64) = 128
    # j=0 (Gre): [Ca; -Sa], j=1 (Gim): [Sa; Ca]
    liw = np.stack([
        np.stack([np.concatenate([Ca[:, 0:128], -Sa[:, 0:128]], axis=0),
                  np.concatenate([Sa[:, 0:128], Ca[:, 0:128]], axis=0)]),
        np.stack([np.concatenate([Ca[:, 128:256], -Sa[:, 128:256]], axis=0),
                  np.concatenate([Sa[:, 128:256], Ca[:, 128:256]], axis=0)]),
    ]).astype(bf)                                                     # (2,2,128,128)
    lih = np.stack([lih_full[:, 0:128], lih_full[:, 128:256]]).astype(bf)  # (2,128,128)
    return ffwd, lbw, liw, lih


def _build_nc(loop_iters=0, probe=None):
    """loop_iters>0 wraps the whole per-batch pipeline in an on-device
    For_i repeat loop — used only by the timing harness to amortize the
    ~80ms axon dispatch overhead out of the measurement.
    probe: None | 'dma' (DMAs only) | 'compute' (no input/residual DMAs)."""
    nc = bacc.Bacc()

    xbf = nc.declare_dram_parameter("xbf", [B, H, W, BS], BF16, isOutput=False)
    ffwd_d = nc.declare_dram_parameter("ffwd", [2, P, P], BF16, isOutput=False)
    lbw_d = nc.declare_dram_parameter("lbw", [2, 2, P, P], BF16, isOutput=False)
    m1_d = nc.declare_dram_parameter("m1", [2, P, HID], BF16, isOutput=False)
    m2_d = nc.declare_dram_parameter("m2", [2, HID, P], BF16, isOutput=False)
    b1s_d = nc.declare_dram_parameter("b1s", [2, HID, 1], F32, isOutput=False)
    b2s_d = nc.declare_dram_parameter("b2s", [P, 1], F32, isOutput=False)
    liw_d = nc.declare_dram_parameter("liw", [2, 2, P, P], BF16, isOutput=False)
    lih_d = nc.declare_dram_parameter("lih", [2, P, P], BF16, isOutput=False)
    out = nc.declare_dram_parameter("out", [B, H, W, BS], BF16, isOutput=True)

    with TileContext(nc) as tc:
        consts = tc.alloc_tile_pool(name="consts", bufs=1)
        ident = consts.tile([P, P], BF16, name="ident")
        masks.make_identity(nc, ident[:])

        def const2d(name, dram_ap, shape, dtype=BF16):
            t = consts.tile(shape, dtype, name=name)
            nc.sync.dma_start(out=t[:], in_=dram_ap)
            return t

        FW = [const2d(f"fw{hh}", ffwd_d[hh], [P, P]) for hh in range(2)]
        LBW = [[const2d(f"lbw{wh}{s}", lbw_d[wh, s], [P, P]) for s in range(2)]
               for wh in range(2)]
        M1 = [const2d(f"m1_{j}", m1_d[j], [P, HID]) for j in range(2)]
        M2 = [const2d(f"m2_{s}", m2_d[s], [HID, P]) for s in range(2)]
        LIW = [[const2d(f"liw{wh}{j}", liw_d[wh, j], [P, P])
                for j in range(2)] for wh in range(2)]
        LIH = [const2d(f"lih{hc}", lih_d[hc], [P, P]) for hc in range(2)]
        b1s_t = [const2d(f"b1s{j}", b1s_d[j], [HID, 1], F32) for j in range(2)]
        b2s_t = const2d("b2s", b2s_d[:], [P, 1], F32)

        # PSUM eviction engine choice (PSUM-capable engines: DVE + ACT).
        # f32-source copies run 1x everywhere -> ACT (1.2 GHz beats DVE
        # 0.96); all-bf16 unit-stride copies hit DVE's 2x packed mode.
        def cp_act(dst, src):
            nc.scalar.activation(out=dst, in_=src, func=AF.Copy)

        def cp_dve(dst, src):
            nc.vector.tensor_copy(out=dst, in_=src)

        # Tag-sharing across stage lifetimes keeps SBUF within budget:
        #   tagA/tagB: U[wh] -> G[wh]   tagC/tagD: V[wh] -> Ght[wh]
        #   tagE: Y -> R                tagF: Yt -> O2
        sb = tc.alloc_tile_pool(name="sb", bufs=1)
        xin = tc.alloc_tile_pool(name="xin", bufs=1)
        outp = tc.alloc_tile_pool(name="outp", bufs=4)
        pmm = tc.alloc_tile_pool(name="pmm", bufs=3, space="PSUM")
        ptp = tc.alloc_tile_pool(name="ptp", bufs=2, space="PSUM")

        import contextlib
        loop_ctx = tc.For_i(0, loop_iters, 1) if loop_iters else contextlib.nullcontext()
        with loop_ctx:
            if probe == "dma":
                _emit_dma_probe(nc, tc, locals())
            else:
                _emit_body(nc, tc, locals(), skip_dma=(probe == "compute"))
        ptp.release()
        pmm.release()
        outp.release()
        xin.release()
        sb.release()
        consts.release()
    nc.compile()
    return nc


def _emit_dma_probe(nc, tc, env):
    """Same DMA traffic as the real kernel (x-in bf16, out bf16), no compute."""
    xbf = env["xbf"]; out = env["out"]
    xin = env["xin"]; outp = env["outp"]
    for b in range(B):
        for wc in range(8):
            for hh in range(2):
                t = xin.tile([P, 32, BS], BF16, tag=f"xin{hh}_{wc}",
                             name=f"pxin{hh}_{b}_{wc}")
                nc.sync.dma_start(
                    out=t[:],
                    in_=xbf[b, hh * P:(hh + 1) * P, wc * 32:(wc + 1) * 32, :])
                ot = outp.tile([P, 32, BS], BF16, tag="ot",
                               name=f"pot_{b}_{hh}_{wc}")
                nc.vector.tensor_copy(out=ot[:], in_=t[:])
                nc.sync.dma_start(
                    out=out[b, hh * P:(hh + 1) * P, wc * 32:(wc + 1) * 32, :],
                    in_=ot[:])


def _emit_body(nc, tc, env, skip_dma=False):
    xbf = env["xbf"]; out = env["out"]
    FW = env["FW"]; LBW = env["LBW"]; M1 = env["M1"]; M2 = env["M2"]
    LIW = env["LIW"]; LIH = env["LIH"]; b1s_t = env["b1s_t"]; b2s_t = env["b2s_t"]
    ident = env["ident"]; cp_act = env["cp_act"]; cp_dve = env["cp_dve"]
    sb = env["sb"]; xin = env["xin"]; outp = env["outp"]
    pmm = env["pmm"]; ptp = env["ptp"]

    for b in range(B):
        # ---------------- stage A: U[wh] (128=k1s, (w 128, c 64)) ----------
        # xin tiles stay resident for the whole batch; iH reads them as the
        # residual.
        xt_all = {}
        U = [sb.tile([P, 128, BS], BF16, tag=f"tagAB{wh}", name=f"U{wh}_{b}")
             for wh in range(2)]
        for wc in range(8):          # w chunks of 32
            for hh in range(2):
                t = xin.tile([P, 32, BS], BF16, tag=f"xin{hh}_{wc}",
                             name=f"xin{hh}_{b}_{wc}")
                if not skip_dma:
                    nc.sync.dma_start(
                        out=t[:],
                        in_=xbf[b, hh * P:(hh + 1) * P, wc * 32:(wc + 1) * 32, :])
                else:
                    nc.sync.dma_start(
                        out=t[0:1, 0:1, :],
                        in_=xbf[b, 0:1, 0:1, :])
                xt_all[(hh, wc)] = t
            for np2 in range(2):     # pairs of N=512 pieces -> FD=1024 evict
                ps = pmm.tile([P, 16, BS], F32, tag="mm", name=f"psA_{b}_{wc}_{np2}")
                for i in range(2):
                    nn = np2 * 2 + i
                    nc.tensor.matmul(ps[:, i * 8:(i + 1) * 8, :], FW[0],
                                     xt_all[(0, wc)][:, nn * 8:(nn + 1) * 8, :],
                                     start=True, stop=False)
                    nc.tensor.matmul(ps[:, i * 8:(i + 1) * 8, :], FW[1],
                                     xt_all[(1, wc)][:, nn * 8:(nn + 1) * 8, :],
                                     start=False, stop=True)
                wg = wc * 4 + np2 * 2   # global 8-w group index (0..31)
                cp_act(U[wg // 16][:, (wg % 16) * 8:(wg % 16) * 8 + 16, :], ps[:])

        # ---------------- turn1: V[wh] (128=w, (c 64, k1s 128)) ------------
        # 4 transposes share one PSUM tile -> single FD=512 eviction.
        V = [sb.tile([P, BS, P], BF16, tag=f"tagCD{wh}", name=f"V{wh}_{b}")
             for wh in range(2)]
        for wh in range(2):
            for cb in range(8):      # blocks of 8 c
                pt = ptp.tile([P, 8, P], BF16, tag="tp", name=f"t1_{b}_{wh}_{cb}")
                for i in range(8):
                    nc.tensor.transpose(pt[:, i, :], U[wh][:, :, cb * 8 + i],
                                        ident[:])
                cp_dve(V[wh][:, cb * 8:(cb + 1) * 8, :], pt[:])

        # ---------------- stage B: Y (128=k2s, (c 64, k1 64)) --------------
        Y = sb.tile([P, BS, KEEP], BF16, tag="tagE", name=f"Y_{b}")
        for np2 in range(4):         # pairs of 8-c chunks -> FD=1024 evict
            ps = pmm.tile([P, 16, KEEP], F32, tag="mm", name=f"psB_{b}_{np2}")
            for i in range(2):
                nn = np2 * 2 + i
                first = True
                for wh in range(2):
                    for s in range(2):   # 0: re cols (k1s 0:64), 1: im cols
                        rhs = V[wh][:, nn * 8:(nn + 1) * 8,
                                    s * KEEP:(s + 1) * KEEP]
                        nc.tensor.matmul(ps[:, i * 8:(i + 1) * 8, :],
                                         LBW[wh][s], rhs,
                                         start=first, stop=(wh == 1 and s == 1))
                        first = False
            cp_act(Y[:, np2 * 16:(np2 + 1) * 16, :], ps[:])

        # ------- turn2: Yt ((s,c) 128, (k1 64, k2 64)) s-stacked -----------
        # Yt[64s+c, k1, k2] = Y[64s+k2, c, k1]; 16 transposes / PSUM tile.
        Yt = sb.tile([P, KEEP, KEEP], BF16, tag="tagF", name=f"Yt_{b}")
        for k1b in range(4):
            pt = ptp.tile([P, 16, KEEP], BF16, tag="tp", name=f"t2_{b}_{k1b}")
            for kk in range(16):
                k1 = k1b * 16 + kk
                for s in range(2):
                    nc.tensor.transpose(
                        pt[s * KEEP:(s + 1) * KEEP, kk, :],
                        Y[s * KEEP:(s + 1) * KEEP, :, k1],
                        ident[s * KEEP:(s + 1) * KEEP, s * KEEP:(s + 1) * KEEP])
            cp_dve(Yt[:, k1b * 16:(k1b + 1) * 16, :], pt[:])

        # ---------------- MLP L1 (K=128) + gelu ----------------------------
        o1 = [sb.tile([HID, KEEP, KEEP], BF16, tag=f"o1_{j}", name=f"o1_{j}_{b}")
              for j in range(2)]
        for j in range(2):
            for k1b in range(4):     # 16 k1 per chunk -> FD=1024 gelu
                ps = pmm.tile([HID, 16, KEEP], F32, tag="mm",
                              name=f"ps1_{b}_{j}_{k1b}")
                for i in range(2):
                    nc.tensor.matmul(
                        ps[:, i * 8:(i + 1) * 8, :], M1[j],
                        Yt[:, k1b * 16 + i * 8:k1b * 16 + (i + 1) * 8, :],
                        start=True, stop=True)
                nc.scalar.activation(out=o1[j][:, k1b * 16:(k1b + 1) * 16, :],
                                     in_=ps[:], func=AF.Gelu, bias=b1s_t[j][:])

        # ---------------- MLP L2 (K=128) + bias ----------------------------
        O2 = sb.tile([P, KEEP, KEEP], BF16, tag="tagF", name=f"O2_{b}")
        for k1b in range(4):
            ps = pmm.tile([P, 16, KEEP], F32, tag="mm", name=f"ps2_{b}_{k1b}")
            for i in range(2):
                sl = slice(k1b * 16 + i * 8, k1b * 16 + (i + 1) * 8)
                nc.tensor.matmul(ps[:, i * 8:(i + 1) * 8, :], M2[0],
                                 o1[0][:, sl, :], start=True, stop=False)
                nc.tensor.matmul(ps[:, i * 8:(i + 1) * 8, :], M2[1],
                                 o1[1][:, sl, :], start=False, stop=True)
            nc.scalar.activation(out=O2[:, k1b * 16:(k1b + 1) * 16, :],
                                 in_=ps[:], func=AF.Identity, bias=b2s_t[:])

        # ------- turn3: R ((s,k2) 128, (k1 64, o 64)) s-stacked ------------
        # R[64s+k2, k1, o] = O2[64s+o, k1, k2]
        R = sb.tile([P, KEEP, KEEP], BF16, tag="tagE", name=f"R_{b}")
        for k1b in range(4):
            pt = ptp.tile([P, 16, KEEP], BF16, tag="tp", name=f"t3_{b}_{k1b}")
            for kk in range(16):
                k1 = k1b * 16 + kk
                for s in range(2):
                    nc.tensor.transpose(
                        pt[s * KEEP:(s + 1) * KEEP, kk, :],
                        O2[s * KEEP:(s + 1) * KEEP, k1, :],
                        ident[s * KEEP:(s + 1) * KEEP, s * KEEP:(s + 1) * KEEP])
            cp_dve(R[:, k1b * 16:(k1b + 1) * 16, :], pt[:])

        # ---------------- invW: G[wh] (128=w, (j 2, k1 64, c' 64)) ---------
        G = [sb.tile([P, 2, KEEP, BS], BF16, tag=f"tagAB{wh}", name=f"G{wh}_{b}")
             for wh in range(2)]
        for wh in range(2):
            for j in range(2):       # 0: Gre, 1: Gim
                for k1b in range(4):
                    ps = pmm.tile([P, 16, BS], F32, tag="mm",
                                  name=f"psW_{b}_{wh}_{j}_{k1b}")
                    for i in range(2):
                        sl = slice(k1b * 16 + i * 8, k1b * 16 + (i + 1) * 8)
                        nc.tensor.matmul(ps[:, i * 8:(i + 1) * 8, :],
                                         LIW[wh][j], R[:, sl, :],
                                         start=True, stop=True)
                    (cp_act if k1b % 2 == 0 else cp_dve)(
                        G[wh][:, j, k1b * 16:(k1b + 1) * 16, :], ps[:])

        # ---------------- turn4: Ght (128=k1s, (w 128, c' 64)) -------------
        # 4 transposes / PSUM tile; eviction is a strided (1x) copy because
        # PSUM holds (c, w) but Ght needs (w, c).
        Ght = [sb.tile([P, P, BS], BF16, tag=f"tagCD{wh}", name=f"Ght{wh}_{b}")
               for wh in range(2)]
        for wh in range(2):
            for cb in range(8):
                pt = ptp.tile([P, 8, P], BF16, tag="tp", name=f"t4_{b}_{wh}_{cb}")
                for i in range(8):
                    # free slice (j 2, k1 64) -> out partitions [k1re | k1im]
                    nc.tensor.transpose(pt[:, i, :],
                                        G[wh][:, :, :, cb * 8 + i], ident[:])
                (cp_dve if cb % 2 == 0 else cp_act)(
                    Ght[wh][:, :, cb * 8:(cb + 1) * 8],
                    pt.transpose([0, 2, 1]))

        # ---------------- invH + residual + store --------------------------
        for q8 in range(8):          # groups of 32 w
            for hc in range(2):
                ot = outp.tile([P, 32, BS], BF16, tag="ot",
                               name=f"ot_{b}_{hc}_{q8}")
                for np2 in range(2):  # pairs of N=512 pieces -> FD=1024
                    ps = pmm.tile([P, 16, BS], F32, tag="mm",
                                  name=f"psH_{b}_{hc}_{q8}_{np2}")
                    for i in range(2):
                        wg = q8 * 4 + np2 * 2 + i   # global 8-w group (0..31)
                        nc.tensor.matmul(
                            ps[:, i * 8:(i + 1) * 8, :], LIH[hc],
                            Ght[wg // 16][:, (wg % 16) * 8:(wg % 16) * 8 + 8, :],
                            start=True, stop=True)
                    osl = ot[:, np2 * 16:(np2 + 1) * 16, :]
                    xsl = xt_all[(hc, q8)][:, np2 * 16:(np2 + 1) * 16, :]
                    # direct PSUM-side add on DVE (f32 src, 1x)
                    nc.vector.tensor_add(out=osl, in0=ps[:], in1=xsl)
                if not skip_dma:
                    nc.sync.dma_start(
                        out=out[b, hc * P:(hc + 1) * P,
                                q8 * 32:(q8 + 1) * 32, :],
                        in_=ot[:])
                else:
                    nc.sync.dma_start(
                        out=out[b, 0:1, 0:1, :],
                        in_=ot[0:1, 0:1, :])


def _prepare_in_maps(x, w1, b1, w2, b2):
    bf = ml_dtypes.bfloat16
    ffwd, lbw, liw, lih = _host_consts()
    x = np.asarray(x, dtype=np.float32)

    in_maps = []
    for n in range(NB):
        xs = np.ascontiguousarray(x[..., n * BS:(n + 1) * BS])
        w1n = np.asarray(w1[:, n], dtype=np.float32)   # (2,64,128)
        w2n = np.asarray(w2[:, n], dtype=np.float32)   # (2,128,64)
        b1n = np.asarray(b1[:, n], dtype=np.float32)   # (2,128)
        b2n = np.asarray(b2[:, n], dtype=np.float32)   # (2,64)
        # s-stacked L1 weights: rows = (s 2, c 64) = 128
        m1 = np.stack([
            np.concatenate([w1n[0], -w1n[1]], axis=0),
            np.concatenate([w1n[1], w1n[0]], axis=0),
        ]).astype(bf)                                   # (2,128,128)
        m2 = np.stack([
            np.concatenate([w2n[0], w2n[1]], axis=1),
            np.concatenate([-w2n[1], w2n[0]], axis=1),
        ]).astype(bf)                                   # (2,128,128)
        in_maps.append({
            "xbf": xs.astype(bf),
            "ffwd": ffwd,
            "lbw": lbw,
            "m1": m1,
            "m2": m2,
            "b1s": b1n[:, :, None].copy(),
            "b2s": np.concatenate([b2n[0], b2n[1]])[:, None].copy(),
            "liw": liw,
            "lih": lih,
        })

    return in_maps


def kernel(x, w1, b1, w2, b2):
    global _CACHED_NC
    if _CACHED_NC is None:
        _CACHED_NC = _build_nc()
    nc = _CACHED_NC
    in_maps = _prepare_in_maps(x, w1, b1, w2, b2)
    res = run_bass_kernel_spmd(nc, in_maps, list(range(NB)))
    return np.concatenate(
        [res.results[i]["out"].astype(np.float32) for i in range(NB)], axis=-1)


# revision 24
# speedup vs baseline: 1.1272x; 1.0472x over previous
"""DPOT2D layer (AFNO-style) Trainium2 kernel.

out = x + irfft2_pad(blockMLP(trunc64(rfft2(x))))   (ortho norm)

Sharding: tensor-parallel over the 8 block-diagonal channel groups — core n
gets channels [n*64, (n+1)*64) and its block's MLP weights. Blocks never mix,
so there is zero cross-core communication.

Per core, every FFT stage is a DFT matmul on the TensorEngine (bf16 operands,
fp32 PSUM accumulation), with PE-transpose corner turns between stages:

  A:  U[k1s,(w,c)]    = F_h^T  @ x          (contract h, 2x128 K-chunks)
  t1: V[w,(c,k1s)]    = corner turn of U    (4 transposes / PSUM bank)
  B:  Y[k2s,(c,k1)]   = DFT_w on complex U  (re/im column accumulation)
  t2: Yt[(s,c),(k1,k2)] = s-stacked corner turn of Y (16 transposes / bank)
  L1: o1 = gelu(M1' Yt + b1)                (K=128 single matmul per tile)
  L2: O2[(o2r|o2i),(k1,k2)] = M2 o1 + b2    (K=128)
  t3: R[(s,k2),(k1,o)] = s-stacked corner turn of O2
  iW: G[w,(j,k1,c)]   = hermitian irfft_w matmuls (K=128 single matmul)
  t4: Ght[k1s,(w,c)]  = corner turn of G
  iH: x'[h,(w,c)]     = E_h^T @ Ght + x_bf16 (DVE tensor add), DMA out bf16

DMA traffic per core: 16.8 MB in (bf16 x, kept resident per batch for the
residual) + 16.8 MB out (bf16, host upcasts to f32).
"""

import numpy as np
import ml_dtypes

import concourse.bass as bass
import concourse.mybir as mybir
from concourse import bacc
from concourse import masks
from concourse.tile import TileContext
from concourse.bass_utils import run_bass_kernel_spmd

B = 2
H = 256
W = 256
C = 512
NB = 8
BS = 64          # channels per block (= per core)
KEEP = 64        # kept modes per spatial dim
HID = 128
P = 128

BF16 = mybir.dt.bfloat16
F32 = mybir.dt.float32
AF = mybir.ActivationFunctionType

_CACHED_NC = None


def _host_consts():
    """DFT matrices shared by all cores (fp32 -> bf16)."""
    h = np.arange(H, dtype=np.float64)[:, None]
    k = np.arange(KEEP, dtype=np.float64)[None, :]
    th = 2.0 * np.pi * h * k / H
    F = np.concatenate([np.cos(th), -np.sin(th)], axis=1) / 16.0      # (256,128)
    Fwre, Fwim = F[:, :KEEP], F[:, KEEP:]
    lb_re = np.concatenate([Fwre, Fwim], axis=1)                      # (256,128)
    lb_im = np.concatenate([-Fwim, Fwre], axis=1)
    alpha = np.where(np.arange(KEEP) == 0, 1.0, 2.0)
    k2 = np.arange(KEEP, dtype=np.float64)[:, None]
    wv = np.arange(W, dtype=np.float64)[None, :]
    tw = 2.0 * np.pi * k2 * wv / W
    Ca = alpha[:, None] * np.cos(tw) / 16.0                           # (64,256)
    Sa = alpha[:, None] * np.sin(tw) / 16.0
    k1 = np.arange(KEEP, dtype=np.float64)[:, None]
    hv = np.arange(H, dtype=np.float64)[None, :]
    tih = 2.0 * np.pi * k1 * hv / H
    Ehc = np.cos(tih) / 16.0                                          # (64,256)
    Ehs = np.sin(tih) / 16.0
    lih_full = np.concatenate([Ehc, -Ehs], axis=0)                    # (128,256)

    bf = ml_dtypes.bfloat16
    ffwd = np.stack([F[0:128], F[128:256]]).astype(bf)                # (2,128,128)
    lbw = np.stack([
        np.stack([lb_re[0:128], lb_im[0:128]]),
        np.stack([lb_re[128:256], lb_im[128:256]]),
    ]).astype(bf)                                                     # (2,2,128,128)
    # s-stacked inverse-W weights: rows = (s 2, k2 64) = 128
    # j=0 (Gre): [Ca; -Sa], j=1 (Gim): [Sa; Ca]
    liw = np.stack([
        np.stack([np.concatenate([Ca[:, 0:128], -Sa[:, 0:128]], axis=0),
                  np.concatenate([Sa[:, 0:128], Ca[:, 0:128]], axis=0)]),
        np.stack([np.concatenate([Ca[:, 128:256], -Sa[:, 128:256]], axis=0),
                  np.concatenate([Sa[:, 128:256], Ca[:, 128:256]], axis=0)]),
    ]).astype(bf)                                                     # (2,2,128,128)
    lih = np.stack([lih_full[:, 0:128], lih_full[:, 128:256]]).astype(bf)  # (2,128,128)
    return ffwd, lbw, liw, lih


def _build_nc(loop_iters=0, probe=None):
    """loop_iters>0 wraps the whole per-batch pipeline in an on-device
    For_i repeat loop — used only by the timing harness to amortize the
    ~80ms axon dispatch overhead out of the measurement.
    probe: None | 'dma' (DMAs only) | 'compute' (no input/residual DMAs)."""
    nc = bacc.Bacc()

    xbf = nc.declare_dram_parameter("xbf", [B, H, W, BS], BF16, isOutput=False)
    ffwd_d = nc.declare_dram_parameter("ffwd", [2, P, P], BF16, isOutput=False)
    lbw_d = nc.declare_dram_parameter("lbw", [2, 2, P, P], BF16, isOutput=False)
    m1_d = nc.declare_dram_parameter("m1", [2, P, HID], BF16, isOutput=False)
    m2_d = nc.declare_dram_parameter("m2", [2, HID, P], BF16, isOutput=False)
    b1s_d = nc.declare_dram_parameter("b1s", [2, HID, 1], F32, isOutput=False)
    b2s_d = nc.declare_dram_parameter("b2s", [P, 1], F32, isOutput=False)
    liw_d = nc.declare_dram_parameter("liw", [2, 2, P, P], BF16, isOutput=False)
    lih_d = nc.declare_dram_parameter("lih", [2, P, P], BF16, isOutput=False)
    out = nc.declare_dram_parameter("out", [B, H, W, BS], BF16, isOutput=True)

    with TileContext(nc) as tc:
        consts = tc.alloc_tile_pool(name="consts", bufs=1)
        ident = consts.tile([P, P], BF16, name="ident")
        masks.make_identity(nc, ident[:])

        def const2d(name, dram_ap, shape, dtype=BF16):
            t = consts.tile(shape, dtype, name=name)
            nc.sync.dma_start(out=t[:], in_=dram_ap)
            return t

        FW = [const2d(f"fw{hh}", ffwd_d[hh], [P, P]) for hh in range(2)]
        LBW = [[const2d(f"lbw{wh}{s}", lbw_d[wh, s], [P, P]) for s in range(2)]
               for wh in range(2)]
        M1 = [const2d(f"m1_{j}", m1_d[j], [P, HID]) for j in range(2)]
        M2 = [const2d(f"m2_{s}", m2_d[s], [HID, P]) for s in range(2)]
        LIW = [[const2d(f"liw{wh}{j}", liw_d[wh, j], [P, P])
                for j in range(2)] for wh in range(2)]
        LIH = [const2d(f"lih{hc}", lih_d[hc], [P, P]) for hc in range(2)]
        b1s_t = [const2d(f"b1s{j}", b1s_d[j], [HID, 1], F32) for j in range(2)]
        b2s_t = const2d("b2s", b2s_d[:], [P, 1], F32)

        # PSUM eviction engine choice (PSUM-capable engines: DVE + ACT).
        # f32-source copies run 1x everywhere -> ACT (1.2 GHz beats DVE
        # 0.96); all-bf16 unit-stride copies hit DVE's 2x packed mode.
        def cp_act(dst, src):
            nc.scalar.activation(out=dst, in_=src, func=AF.Copy)

        def cp_dve(dst, src):
            nc.vector.tensor_copy(out=dst, in_=src)

        # Tag-sharing across stage lifetimes keeps SBUF within budget:
        #   tagA/tagB: U[wh] -> G[wh]   tagC/tagD: V[wh] -> Ght[wh]
        #   tagE: Y -> R                tagF: Yt -> O2
        sb = tc.alloc_tile_pool(name="sb", bufs=1)
        xin = tc.alloc_tile_pool(name="xin", bufs=1)
        outp = tc.alloc_tile_pool(name="outp", bufs=6)
        pmm = tc.alloc_tile_pool(name="pmm", bufs=3, space="PSUM")
        ptp = tc.alloc_tile_pool(name="ptp", bufs=2, space="PSUM")

        import contextlib
        loop_ctx = tc.For_i(0, loop_iters, 1) if loop_iters else contextlib.nullcontext()
        with loop_ctx:
            if probe == "dma":
                _emit_dma_probe(nc, tc, locals())
            else:
                _emit_body(nc, tc, locals(), skip_dma=(probe == "compute"))
        ptp.release()
        pmm.release()
        outp.release()
        xin.release()
        sb.release()
        consts.release()
    nc.compile()
    return nc


def _emit_dma_probe(nc, tc, env):
    """Same DMA traffic as the real kernel (x-in bf16, out bf16), no compute."""
    xbf = env["xbf"]; out = env["out"]
    xin = env["xin"]; outp = env["outp"]
    for b in range(B):
        for wc in range(8):
            for hh in range(2):
                t = xin.tile([P, 32, BS], BF16, tag=f"xin{hh}_{wc}",
                             name=f"pxin{hh}_{b}_{wc}")
                nc.sync.dma_start(
                    out=t[:],
                    in_=xbf[b, hh * P:(hh + 1) * P, wc * 32:(wc + 1) * 32, :])
                ot = outp.tile([P, 32, BS], BF16, tag="ot",
                               name=f"pot_{b}_{hh}_{wc}")
                nc.vector.tensor_copy(out=ot[:], in_=t[:])
                nc.sync.dma_start(
                    out=out[b, hh * P:(hh + 1) * P, wc * 32:(wc + 1) * 32, :],
                    in_=ot[:])


def _emit_body(nc, tc, env, skip_dma=False):
    xbf = env["xbf"]; out = env["out"]
    FW = env["FW"]; LBW = env["LBW"]; M1 = env["M1"]; M2 = env["M2"]
    LIW = env["LIW"]; LIH = env["LIH"]; b1s_t = env["b1s_t"]; b2s_t = env["b2s_t"]
    ident = env["ident"]; cp_act = env["cp_act"]; cp_dve = env["cp_dve"]
    sb = env["sb"]; xin = env["xin"]; outp = env["outp"]
    pmm = env["pmm"]; ptp = env["ptp"]

    for b in range(B):
        # ---------------- stage A: U[wh] (128=k1s, (w 128, c 64)) ----------
        # xin tiles stay resident for the whole batch; iH reads them as the
        # residual.
        xt_all = {}
        U = [sb.tile([P, 128, BS], BF16, tag=f"tagAB{wh}", name=f"U{wh}_{b}")
             for wh in range(2)]
        for wc in range(8):          # w chunks of 32
            for hh in range(2):
                t = xin.tile([P, 32, BS], BF16, tag=f"xin{hh}_{wc}",
                             name=f"xin{hh}_{b}_{wc}")
                if not skip_dma:
                    nc.sync.dma_start(
                        out=t[:],
                        in_=xbf[b, hh * P:(hh + 1) * P, wc * 32:(wc + 1) * 32, :])
                else:
                    nc.sync.dma_start(
                        out=t[0:1, 0:1, :],
                        in_=xbf[b, 0:1, 0:1, :])
                xt_all[(hh, wc)] = t
            for np2 in range(2):     # pairs of N=512 pieces -> FD=1024 evict
                ps = pmm.tile([P, 16, BS], F32, tag="mm", name=f"psA_{b}_{wc}_{np2}")
                for i in range(2):
                    nn = np2 * 2 + i
                    nc.tensor.matmul(ps[:, i * 8:(i + 1) * 8, :], FW[0],
                                     xt_all[(0, wc)][:, nn * 8:(nn + 1) * 8, :],
                                     start=True, stop=False)
                    nc.tensor.matmul(ps[:, i * 8:(i + 1) * 8, :], FW[1],
                                     xt_all[(1, wc)][:, nn * 8:(nn + 1) * 8, :],
                                     start=False, stop=True)
                wg = wc * 4 + np2 * 2   # global 8-w group index (0..31)
                cp_act(U[wg // 16][:, (wg % 16) * 8:(wg % 16) * 8 + 16, :], ps[:])

        # ---------------- turn1: V[wh] (128=w, (c 64, k1s 128)) ------------
        # 4 transposes share one PSUM tile -> single FD=512 eviction.
        V = [sb.tile([P, BS, P], BF16, tag=f"tagCD{wh}", name=f"V{wh}_{b}")
             for wh in range(2)]
        for wh in range(2):
            for cb in range(8):      # blocks of 8 c
                pt = ptp.tile([P, 8, P], BF16, tag="tp", name=f"t1_{b}_{wh}_{cb}")
                for i in range(8):
                    nc.tensor.transpose(pt[:, i, :], U[wh][:, :, cb * 8 + i],
                                        ident[:])
                cp_dve(V[wh][:, cb * 8:(cb + 1) * 8, :], pt[:])

        # ---------------- stage B: Y (128=k2s, (c 64, k1 64)) --------------
        Y = sb.tile([P, BS, KEEP], BF16, tag="tagE", name=f"Y_{b}")
        for np2 in range(4):         # pairs of 8-c chunks -> FD=1024 evict
            ps = pmm.tile([P, 16, KEEP], F32, tag="mm", name=f"psB_{b}_{np2}")
            for i in range(2):
                nn = np2 * 2 + i
                first = True
                for wh in range(2):
                    for s in range(2):   # 0: re cols (k1s 0:64), 1: im cols
                        rhs = V[wh][:, nn * 8:(nn + 1) * 8,
                                    s * KEEP:(s + 1) * KEEP]
                        nc.tensor.matmul(ps[:, i * 8:(i + 1) * 8, :],
                                         LBW[wh][s], rhs,
                                         start=first, stop=(wh == 1 and s == 1))
                        first = False
            cp_act(Y[:, np2 * 16:(np2 + 1) * 16, :], ps[:])

        # ------- turn2: Yt ((s,c) 128, (k1 64, k2 64)) s-stacked -----------
        # Yt[64s+c, k1, k2] = Y[64s+k2, c, k1]; 16 transposes / PSUM tile.
        Yt = sb.tile([P, KEEP, KEEP], BF16, tag="tagF", name=f"Yt_{b}")
        for k1b in range(4):
            pt = ptp.tile([P, 16, KEEP], BF16, tag="tp", name=f"t2_{b}_{k1b}")
            for kk in range(16):
                k1 = k1b * 16 + kk
                for s in range(2):
                    nc.tensor.transpose(
                        pt[s * KEEP:(s + 1) * KEEP, kk, :],
                        Y[s * KEEP:(s + 1) * KEEP, :, k1],
                        ident[s * KEEP:(s + 1) * KEEP, s * KEEP:(s + 1) * KEEP])
            cp_dve(Yt[:, k1b * 16:(k1b + 1) * 16, :], pt[:])

        # ---------------- MLP L1 (K=128) + gelu ----------------------------
        o1 = [sb.tile([HID, KEEP, KEEP], BF16, tag=f"o1_{j}", name=f"o1_{j}_{b}")
              for j in range(2)]
        for j in range(2):
            for k1b in range(4):     # 16 k1 per chunk -> FD=1024 gelu
                ps = pmm.tile([HID, 16, KEEP], F32, tag="mm",
                              name=f"ps1_{b}_{j}_{k1b}")
                for i in range(2):
                    nc.tensor.matmul(
                        ps[:, i * 8:(i + 1) * 8, :], M1[j],
                        Yt[:, k1b * 16 + i * 8:k1b * 16 + (i + 1) * 8, :],
                        start=True, stop=True)
                nc.scalar.activation(out=o1[j][:, k1b * 16:(k1b + 1) * 16, :],
                                     in_=ps[:], func=AF.Gelu, bias=b1s_t[j][:])

        # ---------------- MLP L2 (K=128) + bias ----------------------------
        O2 = sb.tile([P, KEEP, KEEP], BF16, tag="tagF", name=f"O2_{b}")
        for k1b in range(4):
            ps = pmm.tile([P, 16, KEEP], F32, tag="mm", name=f"ps2_{b}_{k1b}")
            for i in range(2):
                sl = slice(k1b * 16 + i * 8, k1b * 16 + (i + 1) * 8)
                nc.tensor.matmul(ps[:, i * 8:(i + 1) * 8, :], M2[0],
                                 o1[0][:, sl, :], start=True, stop=False)
                nc.tensor.matmul(ps[:, i * 8:(i + 1) * 8, :], M2[1],
                                 o1[1][:, sl, :], start=False, stop=True)
            nc.scalar.activation(out=O2[:, k1b * 16:(k1b + 1) * 16, :],
                                 in_=ps[:], func=AF.Identity, bias=b2s_t[:])

        # ------- turn3: R ((s,k2) 128, (k1 64, o 64)) s-stacked ------------
        # R[64s+k2, k1, o] = O2[64s+o, k1, k2]
        R = sb.tile([P, KEEP, KEEP], BF16, tag="tagE", name=f"R_{b}")
        for k1b in range(4):
            pt = ptp.tile([P, 16, KEEP], BF16, tag="tp", name=f"t3_{b}_{k1b}")
            for kk in range(16):
                k1 = k1b * 16 + kk
                for s in range(2):
                    nc.tensor.transpose(
                        pt[s * KEEP:(s + 1) * KEEP, kk, :],
                        O2[s * KEEP:(s + 1) * KEEP, k1, :],
                        ident[s * KEEP:(s + 1) * KEEP, s * KEEP:(s + 1) * KEEP])
            cp_dve(R[:, k1b * 16:(k1b + 1) * 16, :], pt[:])

        # ---------------- invW: G[wh] (128=w, (j 2, k1 64, c' 64)) ---------
        G = [sb.tile([P, 2, KEEP, BS], BF16, tag=f"tagAB{wh}", name=f"G{wh}_{b}")
             for wh in range(2)]
        for wh in range(2):
            for j in range(2):       # 0: Gre, 1: Gim
                for k1b in range(4):
                    ps = pmm.tile([P, 16, BS], F32, tag="mm",
                                  name=f"psW_{b}_{wh}_{j}_{k1b}")
                    for i in range(2):
                        sl = slice(k1b * 16 + i * 8, k1b * 16 + (i + 1) * 8)
                        nc.tensor.matmul(ps[:, i * 8:(i + 1) * 8, :],
                                         LIW[wh][j], R[:, sl, :],
                                         start=True, stop=True)
                    (cp_act if k1b % 2 == 0 else cp_dve)(
                        G[wh][:, j, k1b * 16:(k1b + 1) * 16, :], ps[:])

        # ---------------- turn4: Ght (128=k1s, (w 128, c' 64)) -------------
        # 4 transposes / PSUM tile; eviction is a strided (1x) copy because
        # PSUM holds (c, w) but Ght needs (w, c).
        Ght = [sb.tile([P, P, BS], BF16, tag=f"tagCD{wh}", name=f"Ght{wh}_{b}")
               for wh in range(2)]
        for wh in range(2):
            for cb in range(8):
                pt = ptp.tile([P, 8, P], BF16, tag="tp", name=f"t4_{b}_{wh}_{cb}")
                for i in range(8):
                    # free slice (j 2, k1 64) -> out partitions [k1re | k1im]
                    nc.tensor.transpose(pt[:, i, :],
                                        G[wh][:, :, :, cb * 8 + i], ident[:])
                (cp_dve if cb % 2 == 0 else cp_act)(
                    Ght[wh][:, :, cb * 8:(cb + 1) * 8],
                    pt.transpose([0, 2, 1]))

        # ---------------- invH + residual + store --------------------------
        for q8 in range(8):          # groups of 32 w
            for hc in range(2):
                ot = outp.tile([P, 32, BS], BF16, tag="ot",
                               name=f"ot_{b}_{hc}_{q8}")
                for np2 in range(2):  # pairs of N=512 pieces -> FD=1024
                    ps = pmm.tile([P, 16, BS], F32, tag="mm",
                                  name=f"psH_{b}_{hc}_{q8}_{np2}")
                    for i in range(2):
                        wg = q8 * 4 + np2 * 2 + i   # global 8-w group (0..31)
                        nc.tensor.matmul(
                            ps[:, i * 8:(i + 1) * 8, :], LIH[hc],
                            Ght[wg // 16][:, (wg % 16) * 8:(wg % 16) * 8 + 8, :],
                            start=True, stop=True)
                    osl = ot[:, np2 * 16:(np2 + 1) * 16, :]
                    xsl = xt_all[(hc, q8)][:, np2 * 16:(np2 + 1) * 16, :]
                    if np2 == 0:
                        # direct PSUM-side add on DVE (f32 src, 1x)
                        nc.vector.tensor_add(out=osl, in0=ps[:], in1=xsl)
                    else:
                        # ACT evicts PSUM -> bf16, DVE adds all-bf16 (2x)
                        nc.scalar.activation(out=osl, in_=ps[:], func=AF.Copy)
                        nc.vector.tensor_add(out=osl, in0=osl, in1=xsl)
                if not skip_dma:
                    nc.sync.dma_start(
                        out=out[b, hc * P:(hc + 1) * P,
                                q8 * 32:(q8 + 1) * 32, :],
                        in_=ot[:])
                else:
                    nc.sync.dma_start(
                        out=out[b, 0:1, 0:1, :],
                        in_=ot[0:1, 0:1, :])


def _prepare_in_maps(x, w1, b1, w2, b2):
    bf = ml_dtypes.bfloat16
    ffwd, lbw, liw, lih = _host_consts()
    x = np.asarray(x, dtype=np.float32)

    in_maps = []
    for n in range(NB):
        xs = np.ascontiguousarray(x[..., n * BS:(n + 1) * BS])
        w1n = np.asarray(w1[:, n], dtype=np.float32)   # (2,64,128)
        w2n = np.asarray(w2[:, n], dtype=np.float32)   # (2,128,64)
        b1n = np.asarray(b1[:, n], dtype=np.float32)   # (2,128)
        b2n = np.asarray(b2[:, n], dtype=np.float32)   # (2,64)
        # s-stacked L1 weights: rows = (s 2, c 64) = 128
        m1 = np.stack([
            np.concatenate([w1n[0], -w1n[1]], axis=0),
            np.concatenate([w1n[1], w1n[0]], axis=0),
        ]).astype(bf)                                   # (2,128,128)
        m2 = np.stack([
            np.concatenate([w2n[0], w2n[1]], axis=1),
            np.concatenate([-w2n[1], w2n[0]], axis=1),
        ]).astype(bf)                                   # (2,128,128)
        in_maps.append({
            "xbf": xs.astype(bf),
            "ffwd": ffwd,
            "lbw": lbw,
            "m1": m1,
            "m2": m2,
            "b1s": b1n[:, :, None].copy(),
            "b2s": np.concatenate([b2n[0], b2n[1]])[:, None].copy(),
            "liw": liw,
            "lih": lih,
        })

    return in_maps


def kernel(x, w1, b1, w2, b2):
    global _CACHED_NC
    if _CACHED_NC is None:
        _CACHED_NC = _build_nc()
    nc = _CACHED_NC
    in_maps = _prepare_in_maps(x, w1, b1, w2, b2)
    res = run_bass_kernel_spmd(nc, in_maps, list(range(NB)))
    return np.concatenate(
        [res.results[i]["out"].astype(np.float32) for i in range(NB)], axis=-1)


# revision 25
# speedup vs baseline: 1.1827x; 1.0493x over previous
"""DPOT2D layer (AFNO-style) Trainium2 kernel.

out = x + irfft2_pad(blockMLP(trunc64(rfft2(x))))   (ortho norm)

Sharding: tensor-parallel over the 8 block-diagonal channel groups — core n
gets channels [n*64, (n+1)*64) and its block's MLP weights. Blocks never mix,
so there is zero cross-core communication.

Per core, every FFT stage is a DFT matmul on the TensorEngine (bf16 operands,
fp32 PSUM accumulation), with PE-transpose corner turns between stages:

  A:  U[k1s,(w,c)]    = F_h^T  @ x          (contract h, 2x128 K-chunks)
  t1: V[w,(c,k1s)]    = corner turn of U    (4 transposes / PSUM bank)
  B:  Y[k2s,(c,k1)]   = DFT_w on complex U  (re/im column accumulation)
  t2: Yt[(s,c),(k1,k2)] = s-stacked corner turn of Y (16 transposes / bank)
  L1: o1 = gelu(M1' Yt + b1)                (K=128 single matmul per tile)
  L2: O2[(o2r|o2i),(k1,k2)] = M2 o1 + b2    (K=128)
  t3: R[(s,k2),(k1,o)] = s-stacked corner turn of O2
  iW: G[w,(j,k1,c)]   = hermitian irfft_w matmuls (K=128 single matmul)
  t4: Ght[k1s,(w,c)]  = corner turn of G
  iH: x'[h,(w,c)]     = E_h^T @ Ght + x_bf16 (DVE tensor add), DMA out bf16

DMA traffic per core: 16.8 MB in (bf16 x, kept resident per batch for the
residual) + 16.8 MB out (bf16, host upcasts to f32).
"""

import numpy as np
import ml_dtypes

import concourse.bass as bass
import concourse.mybir as mybir
from concourse import bacc
from concourse import masks
from concourse.tile import TileContext
from concourse.bass_utils import run_bass_kernel_spmd

B = 2
H = 256
W = 256
C = 512
NB = 8
BS = 64          # channels per block (= per core)
KEEP = 64        # kept modes per spatial dim
HID = 128
P = 128

BF16 = mybir.dt.bfloat16
F32 = mybir.dt.float32
AF = mybir.ActivationFunctionType

_CACHED_NC = None


def _host_consts():
    """DFT matrices shared by all cores (fp32 -> bf16)."""
    h = np.arange(H, dtype=np.float64)[:, None]
    k = np.arange(KEEP, dtype=np.float64)[None, :]
    th = 2.0 * np.pi * h * k / H
    F = np.concatenate([np.cos(th), -np.sin(th)], axis=1) / 16.0      # (256,128)
    Fwre, Fwim = F[:, :KEEP], F[:, KEEP:]
    lb_re = np.concatenate([Fwre, Fwim], axis=1)                      # (256,128)
    lb_im = np.concatenate([-Fwim, Fwre], axis=1)
    alpha = np.where(np.arange(KEEP) == 0, 1.0, 2.0)
    k2 = np.arange(KEEP, dtype=np.float64)[:, None]
    wv = np.arange(W, dtype=np.float64)[None, :]
    tw = 2.0 * np.pi * k2 * wv / W
    Ca = alpha[:, None] * np.cos(tw) / 16.0                           # (64,256)
    Sa = alpha[:, None] * np.sin(tw) / 16.0
    k1 = np.arange(KEEP, dtype=np.float64)[:, None]
    hv = np.arange(H, dtype=np.float64)[None, :]
    tih = 2.0 * np.pi * k1 * hv / H
    Ehc = np.cos(tih) / 16.0                                          # (64,256)
    Ehs = np.sin(tih) / 16.0
    lih_full = np.concatenate([Ehc, -Ehs], axis=0)                    # (128,256)

    bf = ml_dtypes.bfloat16
    ffwd = np.stack([F[0:128], F[128:256]]).astype(bf)                # (2,128,128)
    lbw = np.stack([
        np.stack([lb_re[0:128], lb_im[0:128]]),
        np.stack([lb_re[128:256], lb_im[128:256]]),
    ]).astype(bf)                                                     # (2,2,128,128)
    # s-stacked inverse-W weights: rows = (s 2, k2 64) = 128
    # j=0 (Gre): [Ca; -Sa], j=1 (Gim): [Sa; Ca]
    liw = np.stack([
        np.stack([np.concatenate([Ca[:, 0:128], -Sa[:, 0:128]], axis=0),
                  np.concatenate([Sa[:, 0:128], Ca[:, 0:128]], axis=0)]),
        np.stack([np.concatenate([Ca[:, 128:256], -Sa[:, 128:256]], axis=0),
                  np.concatenate([Sa[:, 128:256], Ca[:, 128:256]], axis=0)]),
    ]).astype(bf)                                                     # (2,2,128,128)
    lih = np.stack([lih_full[:, 0:128], lih_full[:, 128:256]]).astype(bf)  # (2,128,128)
    return ffwd, lbw, liw, lih


def _build_nc(loop_iters=0, probe=None):
    """loop_iters>0 wraps the whole per-batch pipeline in an on-device
    For_i repeat loop — used only by the timing harness to amortize the
    ~80ms axon dispatch overhead out of the measurement.
    probe: None | 'dma' (DMAs only) | 'compute' (no input/residual DMAs)."""
    nc = bacc.Bacc()

    xbf = nc.declare_dram_parameter("xbf", [B, H, W, BS], BF16, isOutput=False)
    ffwd_d = nc.declare_dram_parameter("ffwd", [2, P, P], BF16, isOutput=False)
    lbw_d = nc.declare_dram_parameter("lbw", [2, 2, P, P], BF16, isOutput=False)
    m1_d = nc.declare_dram_parameter("m1", [2, P, HID], BF16, isOutput=False)
    m2_d = nc.declare_dram_parameter("m2", [2, HID, P], BF16, isOutput=False)
    b1s_d = nc.declare_dram_parameter("b1s", [2, HID, 1], F32, isOutput=False)
    b2s_d = nc.declare_dram_parameter("b2s", [P, 1], F32, isOutput=False)
    liw_d = nc.declare_dram_parameter("liw", [2, 2, P, P], BF16, isOutput=False)
    lih_d = nc.declare_dram_parameter("lih", [2, P, P], BF16, isOutput=False)
    out = nc.declare_dram_parameter("out", [B, H, W, BS], BF16, isOutput=True)

    with TileContext(nc) as tc:
        consts = tc.alloc_tile_pool(name="consts", bufs=1)
        ident = consts.tile([P, P], BF16, name="ident")
        masks.make_identity(nc, ident[:])

        def const2d(name, dram_ap, shape, dtype=BF16):
            t = consts.tile(shape, dtype, name=name)
            nc.sync.dma_start(out=t[:], in_=dram_ap)
            return t

        FW = [const2d(f"fw{hh}", ffwd_d[hh], [P, P]) for hh in range(2)]
        LBW = [[const2d(f"lbw{wh}{s}", lbw_d[wh, s], [P, P]) for s in range(2)]
               for wh in range(2)]
        M1 = [const2d(f"m1_{j}", m1_d[j], [P, HID]) for j in range(2)]
        M2 = [const2d(f"m2_{s}", m2_d[s], [HID, P]) for s in range(2)]
        LIW = [[const2d(f"liw{wh}{j}", liw_d[wh, j], [P, P])
                for j in range(2)] for wh in range(2)]
        LIH = [const2d(f"lih{hc}", lih_d[hc], [P, P]) for hc in range(2)]
        b1s_t = [const2d(f"b1s{j}", b1s_d[j], [HID, 1], F32) for j in range(2)]
        b2s_t = const2d("b2s", b2s_d[:], [P, 1], F32)

        # PSUM eviction engine choice (PSUM-capable engines: DVE + ACT).
        # f32-source copies run 1x everywhere -> ACT (1.2 GHz beats DVE
        # 0.96); all-bf16 unit-stride copies hit DVE's 2x packed mode.
        def cp_act(dst, src):
            nc.scalar.activation(out=dst, in_=src, func=AF.Copy)

        def cp_dve(dst, src):
            nc.vector.tensor_copy(out=dst, in_=src)

        # Tag-sharing across stage lifetimes keeps SBUF within budget:
        #   tagA/tagB: U[wh] -> G[wh]   tagC/tagD: V[wh] -> Ght[wh]
        #   tagE: Y -> R                tagF: Yt -> O2
        sb = tc.alloc_tile_pool(name="sb", bufs=1)
        xin = tc.alloc_tile_pool(name="xin", bufs=1)
        outp = tc.alloc_tile_pool(name="outp", bufs=4)
        pmm = tc.alloc_tile_pool(name="pmm", bufs=3, space="PSUM")
        ptp = tc.alloc_tile_pool(name="ptp", bufs=2, space="PSUM")

        import contextlib
        loop_ctx = tc.For_i(0, loop_iters, 1) if loop_iters else contextlib.nullcontext()
        with loop_ctx:
            if probe == "dma":
                _emit_dma_probe(nc, tc, locals())
            else:
                _emit_body(nc, tc, locals(), skip_dma=(probe == "compute"))
        ptp.release()
        pmm.release()
        outp.release()
        xin.release()
        sb.release()
        consts.release()
    nc.compile()
    return nc


def _emit_dma_probe(nc, tc, env):
    """Same DMA traffic as the real kernel (x-in bf16, out bf16), no compute."""
    xbf = env["xbf"]; out = env["out"]
    xin = env["xin"]; outp = env["outp"]
    for b in range(B):
        for wc in range(8):
            for hh in range(2):
                t = xin.tile([P, 32, BS], BF16, tag=f"xin{hh}_{wc}",
                             name=f"pxin{hh}_{b}_{wc}")
                nc.sync.dma_start(
                    out=t[:],
                    in_=xbf[b, hh * P:(hh + 1) * P, wc * 32:(wc + 1) * 32, :])
                ot = outp.tile([P, 32, BS], BF16, tag="ot",
                               name=f"pot_{b}_{hh}_{wc}")
                nc.vector.tensor_copy(out=ot[:], in_=t[:])
                nc.sync.dma_start(
                    out=out[b, hh * P:(hh + 1) * P, wc * 32:(wc + 1) * 32, :],
                    in_=ot[:])


def _emit_body(nc, tc, env, skip_dma=False):
    xbf = env["xbf"]; out = env["out"]
    FW = env["FW"]; LBW = env["LBW"]; M1 = env["M1"]; M2 = env["M2"]
    LIW = env["LIW"]; LIH = env["LIH"]; b1s_t = env["b1s_t"]; b2s_t = env["b2s_t"]
    ident = env["ident"]; cp_act = env["cp_act"]; cp_dve = env["cp_dve"]
    sb = env["sb"]; xin = env["xin"]; outp = env["outp"]
    pmm = env["pmm"]; ptp = env["ptp"]

    for b in range(B):
        # ---------------- stage A: U[wh] (128=k1s, (w 128, c 64)) ----------
        # xin tiles stay resident for the whole batch; iH reads them as the
        # residual.
        xt_all = {}
        U = [sb.tile([P, 128, BS], BF16, tag=f"tagAB{wh}", name=f"U{wh}_{b}")
             for wh in range(2)]
        for wc in range(8):          # w chunks of 32
            for hh in range(2):
                t = xin.tile([P, 32, BS], BF16, tag=f"xin{hh}_{wc}",
                             name=f"xin{hh}_{b}_{wc}")
                if not skip_dma:
                    nc.sync.dma_start(
                        out=t[:],
                        in_=xbf[b, hh * P:(hh + 1) * P, wc * 32:(wc + 1) * 32, :])
                else:
                    nc.sync.dma_start(
                        out=t[0:1, 0:1, :],
                        in_=xbf[b, 0:1, 0:1, :])
                xt_all[(hh, wc)] = t
            for np2 in range(2):     # pairs of N=512 pieces -> FD=1024 evict
                ps = pmm.tile([P, 16, BS], F32, tag="mm", name=f"psA_{b}_{wc}_{np2}")
                for i in range(2):
                    nn = np2 * 2 + i
                    nc.tensor.matmul(ps[:, i * 8:(i + 1) * 8, :], FW[0],
                                     xt_all[(0, wc)][:, nn * 8:(nn + 1) * 8, :],
                                     start=True, stop=False)
                    nc.tensor.matmul(ps[:, i * 8:(i + 1) * 8, :], FW[1],
                                     xt_all[(1, wc)][:, nn * 8:(nn + 1) * 8, :],
                                     start=False, stop=True)
                wg = wc * 4 + np2 * 2   # global 8-w group index (0..31)
                cp_act(U[wg // 16][:, (wg % 16) * 8:(wg % 16) * 8 + 16, :], ps[:])

        # ---------------- turn1: V[wh] (128=w, (c 64, k1s 128)) ------------
        # 4 transposes share one PSUM tile -> single FD=512 eviction.
        V = [sb.tile([P, BS, P], BF16, tag=f"tagCD{wh}", name=f"V{wh}_{b}")
             for wh in range(2)]
        for wh in range(2):
            for cb in range(8):      # blocks of 8 c
                pt = ptp.tile([P, 8, P], BF16, tag="tp", name=f"t1_{b}_{wh}_{cb}")
                for i in range(8):
                    nc.tensor.transpose(pt[:, i, :], U[wh][:, :, cb * 8 + i],
                                        ident[:])
                cp_dve(V[wh][:, cb * 8:(cb + 1) * 8, :], pt[:])

        # ---------------- stage B: Y (128=k2s, (c 64, k1 64)) --------------
        Y = sb.tile([P, BS, KEEP], BF16, tag="tagE", name=f"Y_{b}")
        for np2 in range(4):         # pairs of 8-c chunks -> FD=1024 evict
            ps = pmm.tile([P, 16, KEEP], F32, tag="mm", name=f"psB_{b}_{np2}")
            for i in range(2):
                nn = np2 * 2 + i
                first = True
                for wh in range(2):
                    for s in range(2):   # 0: re cols (k1s 0:64), 1: im cols
                        rhs = V[wh][:, nn * 8:(nn + 1) * 8,
                                    s * KEEP:(s + 1) * KEEP]
                        nc.tensor.matmul(ps[:, i * 8:(i + 1) * 8, :],
                                         LBW[wh][s], rhs,
                                         start=first, stop=(wh == 1 and s == 1))
                        first = False
            cp_act(Y[:, np2 * 16:(np2 + 1) * 16, :], ps[:])

        # ------- turn2: Yt ((s,c) 128, (k1 64, k2 64)) s-stacked -----------
        # Yt[64s+c, k1, k2] = Y[64s+k2, c, k1]; 16 transposes / PSUM tile.
        Yt = sb.tile([P, KEEP, KEEP], BF16, tag="tagF", name=f"Yt_{b}")
        for k1b in range(4):
            pt = ptp.tile([P, 16, KEEP], BF16, tag="tp", name=f"t2_{b}_{k1b}")
            for kk in range(16):
                k1 = k1b * 16 + kk
                for s in range(2):
                    nc.tensor.transpose(
                        pt[s * KEEP:(s + 1) * KEEP, kk, :],
                        Y[s * KEEP:(s + 1) * KEEP, :, k1],
                        ident[s * KEEP:(s + 1) * KEEP, s * KEEP:(s + 1) * KEEP])
            cp_dve(Yt[:, k1b * 16:(k1b + 1) * 16, :], pt[:])

        # ---------------- MLP L1 (K=128) + gelu ----------------------------
        o1 = [sb.tile([HID, KEEP, KEEP], BF16, tag=f"o1_{j}", name=f"o1_{j}_{b}")
              for j in range(2)]
        for j in range(2):
            for k1b in range(4):     # 16 k1 per chunk -> FD=1024 gelu
                ps = pmm.tile([HID, 16, KEEP], F32, tag="mm",
                              name=f"ps1_{b}_{j}_{k1b}")
                for i in range(2):
                    nc.tensor.matmul(
                        ps[:, i * 8:(i + 1) * 8, :], M1[j],
                        Yt[:, k1b * 16 + i * 8:k1b * 16 + (i + 1) * 8, :],
                        start=True, stop=True)
                nc.scalar.activation(out=o1[j][:, k1b * 16:(k1b + 1) * 16, :],
                                     in_=ps[:], func=AF.Gelu, bias=b1s_t[j][:])

        # ---------------- MLP L2 (K=128) + bias ----------------------------
        O2 = sb.tile([P, KEEP, KEEP], BF16, tag="tagF", name=f"O2_{b}")
        for k1b in range(4):
            ps = pmm.tile([P, 16, KEEP], F32, tag="mm", name=f"ps2_{b}_{k1b}")
            for i in range(2):
                sl = slice(k1b * 16 + i * 8, k1b * 16 + (i + 1) * 8)
                nc.tensor.matmul(ps[:, i * 8:(i + 1) * 8, :], M2[0],
                                 o1[0][:, sl, :], start=True, stop=False)
                nc.tensor.matmul(ps[:, i * 8:(i + 1) * 8, :], M2[1],
                                 o1[1][:, sl, :], start=False, stop=True)
            nc.scalar.activation(out=O2[:, k1b * 16:(k1b + 1) * 16, :],
                                 in_=ps[:], func=AF.Identity, bias=b2s_t[:])

        # ------- turn3: R ((s,k2) 128, (k1 64, o 64)) s-stacked ------------
        # R[64s+k2, k1, o] = O2[64s+o, k1, k2]
        R = sb.tile([P, KEEP, KEEP], BF16, tag="tagE", name=f"R_{b}")
        for k1b in range(4):
            pt = ptp.tile([P, 16, KEEP], BF16, tag="tp", name=f"t3_{b}_{k1b}")
            for kk in range(16):
                k1 = k1b * 16 + kk
                for s in range(2):
                    nc.tensor.transpose(
                        pt[s * KEEP:(s + 1) * KEEP, kk, :],
                        O2[s * KEEP:(s + 1) * KEEP, k1, :],
                        ident[s * KEEP:(s + 1) * KEEP, s * KEEP:(s + 1) * KEEP])
            cp_dve(R[:, k1b * 16:(k1b + 1) * 16, :], pt[:])

        # ---------------- invW: G[wh] (128=w, (j 2, k1 64, c' 64)) ---------
        G = [sb.tile([P, 2, KEEP, BS], BF16, tag=f"tagAB{wh}", name=f"G{wh}_{b}")
             for wh in range(2)]
        for wh in range(2):
            for j in range(2):       # 0: Gre, 1: Gim
                for k1b in range(4):
                    ps = pmm.tile([P, 16, BS], F32, tag="mm",
                                  name=f"psW_{b}_{wh}_{j}_{k1b}")
                    for i in range(2):
                        sl = slice(k1b * 16 + i * 8, k1b * 16 + (i + 1) * 8)
                        nc.tensor.matmul(ps[:, i * 8:(i + 1) * 8, :],
                                         LIW[wh][j], R[:, sl, :],
                                         start=True, stop=True)
                    (cp_act if k1b % 2 == 0 else cp_dve)(
                        G[wh][:, j, k1b * 16:(k1b + 1) * 16, :], ps[:])

        # ---------------- turn4: Ght (128=k1s, (w 128, c' 64)) -------------
        # 4 transposes / PSUM tile; eviction is a strided (1x) copy because
        # PSUM holds (c, w) but Ght needs (w, c).
        Ght = [sb.tile([P, P, BS], BF16, tag=f"tagCD{wh}", name=f"Ght{wh}_{b}")
               for wh in range(2)]
        for wh in range(2):
            for cb in range(8):
                pt = ptp.tile([P, 8, P], BF16, tag="tp", name=f"t4_{b}_{wh}_{cb}")
                for i in range(8):
                    # free slice (j 2, k1 64) -> out partitions [k1re | k1im]
                    nc.tensor.transpose(pt[:, i, :],
                                        G[wh][:, :, :, cb * 8 + i], ident[:])
                (cp_dve if cb % 2 == 0 else cp_act)(
                    Ght[wh][:, :, cb * 8:(cb + 1) * 8],
                    pt.transpose([0, 2, 1]))

        # ---------------- invH + residual + store --------------------------
        for q8 in range(8):          # groups of 32 w
            for hc in range(2):
                ot = outp.tile([P, 32, BS], BF16, tag="ot",
                               name=f"ot_{b}_{hc}_{q8}")
                for np2 in range(2):  # pairs of N=512 pieces -> FD=1024
                    ps = pmm.tile([P, 16, BS], F32, tag="mm",
                                  name=f"psH_{b}_{hc}_{q8}_{np2}")
                    for i in range(2):
                        wg = q8 * 4 + np2 * 2 + i   # global 8-w group (0..31)
                        nc.tensor.matmul(
                            ps[:, i * 8:(i + 1) * 8, :], LIH[hc],
                            Ght[wg // 16][:, (wg % 16) * 8:(wg % 16) * 8 + 8, :],
                            start=True, stop=True)
                    osl = ot[:, np2 * 16:(np2 + 1) * 16, :]
                    xsl = xt_all[(hc, q8)][:, np2 * 16:(np2 + 1) * 16, :]
                    if np2 == 0:
                        # direct PSUM-side add on DVE (f32 src, 1x)
                        nc.vector.tensor_add(out=osl, in0=ps[:], in1=xsl)
                    else:
                        # ACT evicts PSUM -> bf16, DVE adds all-bf16 (2x)
                        nc.scalar.activation(out=osl, in_=ps[:], func=AF.Copy)
                        nc.vector.tensor_add(out=osl, in0=osl, in1=xsl)
                if not skip_dma:
                    nc.sync.dma_start(
                        out=out[b, hc * P:(hc + 1) * P,
                                q8 * 32:(q8 + 1) * 32, :],
                        in_=ot[:])
                else:
                    nc.sync.dma_start(
                        out=out[b, 0:1, 0:1, :],
                        in_=ot[0:1, 0:1, :])


def _prepare_in_maps(x, w1, b1, w2, b2):
    bf = ml_dtypes.bfloat16
    ffwd, lbw, liw, lih = _host_consts()
    x = np.asarray(x, dtype=np.float32)

    in_maps = []
    for n in range(NB):
        xs = np.ascontiguousarray(x[..., n * BS:(n + 1) * BS])
        w1n = np.asarray(w1[:, n], dtype=np.float32)   # (2,64,128)
        w2n = np.asarray(w2[:, n], dtype=np.float32)   # (2,128,64)
        b1n = np.asarray(b1[:, n], dtype=np.float32)   # (2,128)
        b2n = np.asarray(b2[:, n], dtype=np.float32)   # (2,64)
        # s-stacked L1 weights: rows = (s 2, c 64) = 128
        m1 = np.stack([
            np.concatenate([w1n[0], -w1n[1]], axis=0),
            np.concatenate([w1n[1], w1n[0]], axis=0),
        ]).astype(bf)                                   # (2,128,128)
        m2 = np.stack([
            np.concatenate([w2n[0], w2n[1]], axis=1),
            np.concatenate([-w2n[1], w2n[0]], axis=1),
        ]).astype(bf)                                   # (2,128,128)
        in_maps.append({
            "xbf": xs.astype(bf),
            "ffwd": ffwd,
            "lbw": lbw,
            "m1": m1,
            "m2": m2,
            "b1s": b1n[:, :, None].copy(),
            "b2s": np.concatenate([b2n[0], b2n[1]])[:, None].copy(),
            "liw": liw,
            "lih": lih,
        })

    return in_maps


def kernel(x, w1, b1, w2, b2):
    global _CACHED_NC
    if _CACHED_NC is None:
        _CACHED_NC = _build_nc()
    nc = _CACHED_NC
    in_maps = _prepare_in_maps(x, w1, b1, w2, b2)
    res = run_bass_kernel_spmd(nc, in_maps, list(range(NB)))
    return np.concatenate(
        [res.results[i]["out"].astype(np.float32) for i in range(NB)], axis=-1)
